# revision 1
# baseline (speedup 1.0000x reference)
"""Trainium2 Bass kernel for nn_BasicConv_78915729097031 (e3nn-style GNN conv).

Math per edge e (i=src, j=dst):
    w_e   = radial_mlp(emb_e)                # [4096] per-edge TP weights
    msg_e = TP(x[i_e], sh_e, w_e)            # [128]
    out[n] = (1/sqrt(8)) * sum_{e: j_e=n} msg_e

Strategy (8 NeuronCores, SPMD, no collectives):
  * Host: sort edges by destination, split the 8192 nodes into 64 chunks of
    128; each core owns 8 chunks (1024 nodes) and the edges targeting them,
    padded per-chunk to a multiple of 128 edges.
  * Host precomputes the six per-edge 32-vectors ("a-vectors") that the
    Clebsch-Gordan contraction needs (x0*sh0, dot(x1,sh1), x0, x1_k*sh0) with
    all scalar norms folded in, so the device never gathers x at all.
  * Device per 128-edge tile: MLP1 (PE, fp32) -> silu (ACT) -> MLP2
    (PE, f32r full-rate, vs stationary h^T) -> per-edge TP contraction as a
    single fused DVE pass (custom multiply+cumsum op reading PSUM, writing
    only per-w running sums; per-w sums recovered by differencing on GPSIMD)
    -> segment-sum into the owning node chunk via one-hot matmuls (PE, fp32,
    PSUM-accumulated across the chunk).
  * Output: each core writes its 1024 node rows; host concatenates and
    reorders out1 columns to the reference (w*3+k) interleave.
"""
import os
import sys

import numpy as np

for _p in ("/opt/trn_rl_repo", "/root/.axon_site/_ro/trn_rl_repo"):
    if os.path.isdir(_p) and _p not in sys.path:
        sys.path.insert(0, _p)
        break

MUL = 32
N_NODES = 8192
N_EDGES = 65536
INV_SQRT3 = 1.0 / np.sqrt(3.0)
NORM0 = np.sqrt(1.0 / (2.0 * MUL))
NORM1 = np.sqrt(3.0 / (2.0 * MUL))
SILU_GAIN = 1.6790
NUM_NEIGHBORS = 8.0
NC = 8
NPC = 128                          # nodes per chunk
CHUNKS_PER_CORE = (N_NODES // NPC) // NC   # 8

# MLP2 matmul operand dtype: "f32r" (full-rate fp32 storage) | "bf16" | "f32"
MM2_MODE = os.environ.get("KERNEL_MM2_MODE", "f32r")


# --------------------------------------------------------------------------- #
# Host-side preparation
# --------------------------------------------------------------------------- #
def _host_prep(x, edge_index, edge_attr, edge_len_emb, W1, W2):
    i = edge_index[0].astype(np.int64)
    j = edge_index[1].astype(np.int64)
    E = i.shape[0]
    order = np.argsort(j, kind="stable")
    i_s, j_s = i[order], j[order]
    sh = edge_attr[order].astype(np.float32)
    emb = edge_len_emb[order].astype(np.float32)
    xg = x[i_s].astype(np.float32)
    x0 = xg[:, :MUL]
    x1 = xg[:, MUL:].reshape(E, MUL, 3)
    sh0 = sh[:, 0]
    sh1 = sh[:, 1:4]

    s8 = 1.0 / np.sqrt(NUM_NEIGHBORS)
    # unit order: [a, d0, d1, d2, b, c]  (so that [a|d0|d1|d2] is one
    # contiguous 128-col block feeding the full-width segment matmul)
    av = np.zeros((E, 6, MUL), np.float32)
    av[:, 0] = x0 * sh0[:, None] * (NORM0 * s8)
    for k in range(3):
        av[:, 1 + k] = x1[:, :, k] * (sh0[:, None] * (INV_SQRT3 * NORM1 * s8))
    av[:, 4] = np.einsum("eui,ei->eu", x1, sh1) * (INV_SQRT3 * NORM0 * s8)
    av[:, 5] = x0 * (INV_SQRT3 * NORM1 * s8)

    W1eff = (W1 / np.sqrt(W1.shape[0])).astype(np.float32)              # [64,128]
    W2eff = (SILU_GAIN * W2 / np.sqrt(W2.shape[0])).astype(np.float32)  # [128,4096]
    # quarter q = path block; within a quarter lay out (w outer, u inner) so
    # the TP contraction index u is innermost in the PSUM stream.
    W2eff = (W2eff.reshape(128, 4, MUL, MUL)      # [h, path, u, w]
             .transpose(0, 1, 3, 2)               # [h, path, w, u]
             .reshape(128, 4096).copy())

    n_chunks = N_NODES // NPC
    chunk_of_edge = j_s // NPC
    counts = np.bincount(chunk_of_edge, minlength=n_chunks)
    tiles_of_chunk = np.maximum(1, np.ceil(counts / 128).astype(np.int64))

    # snake-pack chunks onto cores by descending tile count; per-slot
    # schedule = max across cores, shared by the SPMD program
    order = np.argsort(-tiles_of_chunk, kind="stable")
    assign = np.empty((NC, CHUNKS_PER_CORE), np.int64)   # (core, slot) -> chunk
    for s in range(CHUNKS_PER_CORE):
        row = order[s * NC:(s + 1) * NC]
        assign[:, s] = row if s % 2 == 0 else row[::-1]
    schedule = tuple(int(tiles_of_chunk[assign[:, s]].max())
                     for s in range(CHUNKS_PER_CORE))
    slot_base = np.concatenate([[0], np.cumsum(np.array(schedule) * 128)])
    e_pad = int(slot_base[-1])
    n_tiles = sum(schedule)

    # emb stored per-tile contiguous: [n_tiles, 64, 128]
    embT = np.zeros((NC, n_tiles, 64, 128), np.float32)
    # av_ext = [a|d0|d1|d2|b|c (192) | nloc (1) | sh1 (3)]
    avx = np.zeros((NC, e_pad, 196), np.float32)
    avx[:, :, 192] = -1.0
    starts = np.concatenate([[0], np.cumsum(counts)])
    for core in range(NC):
        for s in range(CHUNKS_PER_CORE):
            c = int(assign[core, s])
            lo, hi = int(starts[c]), int(starts[c + 1])
            cnt = hi - lo
            base = int(slot_base[s])
            et = emb[lo:hi].T                      # [64, cnt]
            etp = np.zeros((64, schedule[s] * 128), np.float32)
            etp[:, :cnt] = et
            t0 = sum(schedule[:s])
            embT[core, t0:t0 + schedule[s]] = (
                etp.reshape(64, schedule[s], 128).transpose(1, 0, 2))
            avx[core, base:base + cnt, :192] = av[lo:hi].reshape(cnt, -1)
            avx[core, base:base + cnt, 192] = (j_s[lo:hi] - c * NPC).astype(np.float32)
            avx[core, base:base + cnt, 193:196] = sh1[lo:hi]
    return dict(embT=embT, avx=avx, W1eff=W1eff, W2eff=W2eff,
                schedule=schedule, e_pad=e_pad, n_tiles=n_tiles,
                assign=assign)


# --------------------------------------------------------------------------- #
# Bass program
# --------------------------------------------------------------------------- #
_PROGRAM_CACHE = {}

_SCAN_OP_NAME = "TT_MUL_CUMSUM_ANT"


def _register_scan_op():
    """Custom DVE op: out = running cumsum of Src0*Src1 along the free dim.
    Called with a step-0 inner output AP it writes only the 32 per-page
    boundary values (page = one w, 32 u-terms)."""
    import concourse.dve_ops as dve_ops
    for o in dve_ops.OPS:
        if o.name == _SCAN_OP_NAME:
            return o
    import numpy as np
    from concourse.dve_spec import Spec, Src0, Src1, scan, AluOp, lower, _has_src1
    from concourse.dve_uop import DveOpSpec

    def _ref(in0, in1, s0, s1, imm2):
        prod = in0.astype(np.float32) * in1.astype(np.float32)
        flat = prod.reshape(prod.shape[0], -1)
        return np.cumsum(flat, axis=-1).reshape(prod.shape)

    spec = Spec(body=scan(AluOp.ADD, Src0 * Src1), reference=_ref)
    shas = {}
    for ver in ("v3", "v4"):
        tmp = DveOpSpec(name=_SCAN_OP_NAME, opcode=0, uops=lower(spec, ver=ver),
                        rd1_en=_has_src1(spec))
        shas[ver] = tmp.sha(ver)
    op = dve_ops.DveOp(_SCAN_OP_NAME, spec, subdim=True, uops_sha=shas)
    dve_ops.OPS.append(op)
    dve_ops._SUB_OPCODE_FOR_NAME[_SCAN_OP_NAME] = (
        dve_ops._CUSTOM_DVE_ROW_BASE + len(dve_ops.OPS) - 1)
    dve_ops.CUSTOM_DVE_SPECS[_SCAN_OP_NAME] = spec
    return op


def _build_program(schedule, e_pad, repeat=1):
    key = (schedule, e_pad, MM2_MODE, repeat)
    if key in _PROGRAM_CACHE:
        return _PROGRAM_CACHE[key]

    from concourse import bacc, bass, mybir
    import concourse.tile as tile

    scan_op = _register_scan_op()

    f32 = mybir.dt.float32
    bf16 = mybir.dt.bfloat16
    f32r = mybir.dt.float32r
    AF = mybir.ActivationFunctionType
    OP = mybir.AluOpType

    w2_dt = {"f32r": f32, "bf16": bf16, "f32": f32}[MM2_MODE]
    h_dt = {"f32r": f32r, "bf16": bf16, "f32": f32}[MM2_MODE]

    nc = bacc.Bacc("TRN2", target_bir_lowering=False, debug=False, num_devices=NC)

    n_tiles = sum(schedule)
    embT_d = nc.dram_tensor("embT", [n_tiles, 64, 128], f32,
                            kind="ExternalInput").ap()
    avx_d = nc.dram_tensor("avx", [e_pad, 196], f32, kind="ExternalInput").ap()
    w1_d = nc.dram_tensor("w1", [64, 128], f32, kind="ExternalInput").ap()
    w2_d = nc.dram_tensor("w2", [128, 4096], w2_dt, kind="ExternalInput").ap()
    iota_d = nc.dram_tensor("iota", [128, 128], f32, kind="ExternalInput").ap()
    out_d = nc.dram_tensor("out", [CHUNKS_PER_CORE * 128, 128], f32,
                           kind="ExternalOutput").ap()

    with tile.TileContext(nc) as tc:
        with (
            tc.tile_pool(name="const", bufs=1) as const_p,
            tc.tile_pool(name="inp", bufs=4) as inp_p,
            tc.tile_pool(name="hsb", bufs=4) as h_p,
            tc.tile_pool(name="red", bufs=4) as red_p,
            tc.tile_pool(name="sel", bufs=3) as sel_p,
            tc.tile_pool(name="osb", bufs=2) as out_p,
            tc.tile_pool(name="hps", bufs=1, space="PSUM") as hps_p,
            tc.tile_pool(name="wps", bufs=3, space="PSUM") as wps_p,
            tc.tile_pool(name="mps", bufs=1, space="PSUM") as mps_p,
        ):
            w1_sb = const_p.tile([64, 128], f32)
            nc.sync.dma_start(w1_sb[:], w1_d[:])
            w2_sb = const_p.tile([128, 4096], w2_dt)
            nc.sync.dma_start(w2_sb[:], w2_d[:])
            if MM2_MODE == "f32r":
                # f32r matmul operands must come from an op that rounds to f32r
                w2_r = const_p.tile([128, 4096], f32r)
                nc.vector.tensor_copy(out=w2_r[:], in_=w2_sb[:])
                w2_mm = w2_r
            else:
                w2_mm = w2_sb
            iota_sb = const_p.tile([128, 128], f32)
            nc.sync.dma_start(iota_sb[:], iota_d[:])


            for cc_rep in range(CHUNKS_PER_CORE * repeat):
                cc = cc_rep % CHUNKS_PER_CORE
                m_ps = mps_p.tile([128, 128], f32, space="PSUM", tag="m")
                tpc = schedule[cc]
                t_base = sum(schedule[:cc])
                for t in range(tpc):
                    til = t_base + t
                    e0 = til * 128
                    first, last = t == 0, t == tpc - 1

                    # ---- loads ----
                    emb_sb = inp_p.tile([64, 128], f32, tag="emb")
                    nc.sync.dma_start(emb_sb[:], embT_d[til])
                    av_sb = inp_p.tile([128, 196], f32, tag="av")
                    nc.sync.dma_start(av_sb[:], avx_d[e0:e0 + 128, :])
                    nloc_sb = av_sb[:, 192:193]
                    sh1_sb = av_sb[:, 193:196]

                    # ---- MLP1 (fp32 PE) + silu -> h^T tile [128h, 128e] ----
                    hpre = hps_p.tile([128, 128], f32, space="PSUM", tag="hpre")
                    nc.tensor.matmul(hpre[:], lhsT=w1_sb[:], rhs=emb_sb[:],
                                     start=True, stop=True)
                    h_sb = h_p.tile([128, 128], h_dt, tag="h")
                    nc.scalar.activation(h_sb[:], hpre[:], AF.Silu)

                    # ---- MLP2: w tile = h^T.T @ W2 -> 4 PSUM quarters ----
                    wq = []
                    for q in range(4):
                        wq_ps = wps_p.tile([128, 1024], f32, space="PSUM",
                                           tag="wq")
                        for half in range(2):
                            sl = slice(half * 512, half * 512 + 512)
                            nc.tensor.matmul(
                                wq_ps[:, sl],
                                lhsT=h_sb[:],
                                rhs=w2_mm[:, q * 1024 + half * 512:
                                          q * 1024 + half * 512 + 512],
                                start=True, stop=True)
                        wq.append(wq_ps)

                    # ---- TP: fused multiply + cumsum over u (custom DVE op).
                    # cum[:, q, w] = running sum through page w of unit q's
                    # stream; per-w sums recovered by differencing.
                    # cum unit order: [a, d0, d1, d2, b, c]; PSUM quarter
                    # feeding each: [0, 3, 3, 3, 1, 2].
                    # d quarter is read 3x: stage it to SBUF once on the idle
                    # scalar engine (cheaper DVE startup + frees the PSUM slot)
                    d_sb = red_p.tile([128, 1024], f32, tag="dcp")
                    nc.scalar.copy(out=d_sb[:], in_=wq[3][:])
                    # scan order releases PSUM quarters a,b,c as early as
                    # possible so the next tile's MLP2 can reuse their slots
                    cum = red_p.tile([128, 6, 32], f32, tag="cum")
                    for q, wsrc in ((0, 0), (4, 1), (5, 2), (1, 3), (2, 3), (3, 3)):
                        src = d_sb if wsrc == 3 else wq[wsrc]
                        nc.vector._custom_dve(
                            scan_op,
                            out=cum[:, q].rearrange("p w -> p w ()")
                                .to_broadcast([128, 32, 32]),
                            in0=src[:].rearrange("p (w u) -> p w u", w=32),
                            in1=av_sb[:, q * 32:(q + 1) * 32]
                                .rearrange("p u -> p () u")
                                .to_broadcast([128, 32, 32]),
                        )
                    # per-w sums = diffs of the running cums (each of the 6
                    # calls starts fresh at flat position q*32)
                    red = red_p.tile([128, 6, 32], f32, tag="red")
                    nc.gpsimd.tensor_copy(out=red[:, :, 0:1], in_=cum[:, :, 0:1])
                    nc.gpsimd.tensor_tensor(out=red[:, :, 1:32],
                                            in0=cum[:, :, 1:32],
                                            in1=cum[:, :, 0:31],
                                            op=OP.subtract)

                    # ---- one-hot select matrices (GPSIMD) ----
                    s_sb = sel_p.tile([128, 128], f32, tag="s")
                    nc.gpsimd.tensor_scalar(out=s_sb[:], in0=iota_sb[:],
                                            scalar1=nloc_sb[:, 0:1], scalar2=None,
                                            op0=OP.is_equal)
                    sck = sel_p.tile([128, 3, 128], f32, tag="sck")
                    for k in range(3):
                        nc.gpsimd.tensor_scalar(out=sck[:, k], in0=s_sb[:],
                                                scalar1=sh1_sb[:, k:k + 1],
                                                scalar2=None, op0=OP.mult)

                    # ---- segment matmuls (accumulate across chunk) ----
                    red_f = red[:].rearrange("p q w -> p (q w)")
                    # [a|d0|d1|d2] block -> all 128 out cols (clears PSUM on
                    # the chunk's first tile)
                    nc.tensor.matmul(m_ps[:], lhsT=s_sb[:], rhs=red_f[:, 0:128],
                                     start=first, stop=False,
                                     skip_group_check=True)
                    # b adds into out0
                    nc.tensor.matmul(m_ps[:, 0:32], lhsT=s_sb[:],
                                     rhs=red_f[:, 128:160],
                                     start=False, stop=False,
                                     skip_group_check=True)
                    # c * sh1_k adds into out1_k
                    for k in range(3):
                        nc.tensor.matmul(m_ps[:, 32 + 32 * k:64 + 32 * k],
                                         lhsT=sck[:, k], rhs=red_f[:, 160:192],
                                         start=False,
                                         stop=(last and k == 2),
                                         skip_group_check=True)

                # ---- store chunk ----
                o_sb = out_p.tile([128, 128], f32, tag="o")
                nc.scalar.copy(out=o_sb[:], in_=m_ps[:])
                nc.sync.dma_start(out_d[cc * 128:(cc + 1) * 128, :], o_sb[:])

    nc.compile()
    _PROGRAM_CACHE[key] = nc
    return nc


# --------------------------------------------------------------------------- #
# Entry point
# --------------------------------------------------------------------------- #
def _build_in_maps(prep):
    iota = np.broadcast_to(np.arange(128, dtype=np.float32), (128, 128)).copy()
    if MM2_MODE == "bf16":
        import ml_dtypes
        w2_up = prep["W2eff"].astype(ml_dtypes.bfloat16)
    else:
        w2_up = prep["W2eff"]
    in_maps = []
    for c in range(NC):
        in_maps.append({
            "embT": prep["embT"][c],
            "avx": prep["avx"][c],
            "w1": prep["W1eff"],
            "w2": w2_up,
            "iota": iota,
        })
    return in_maps


def _postprocess(per_core_out, assign):
    M = np.empty((N_NODES, 128), np.float32)
    for core in range(NC):
        for s in range(CHUNKS_PER_CORE):
            c = int(assign[core, s])
            M[c * NPC:(c + 1) * NPC] = per_core_out[core][s * NPC:(s + 1) * NPC]
    out = np.empty((N_NODES, 128), np.float32)
    out[:, :32] = M[:, :32]
    out[:, 32:] = (M[:, 32:].reshape(N_NODES, 3, 32)
                   .transpose(0, 2, 1).reshape(N_NODES, 96))
    return out


def _prepare(x, edge_index, edge_attr, edge_len_emb, W1, W2, repeat=1):
    x = np.asarray(x, np.float32)
    edge_index = np.asarray(edge_index)
    edge_attr = np.asarray(edge_attr, np.float32)
    edge_len_emb = np.asarray(edge_len_emb, np.float32)
    W1 = np.asarray(W1, np.float32)
    W2 = np.asarray(W2, np.float32)
    prep = _host_prep(x, edge_index, edge_attr, edge_len_emb, W1, W2)
    nc = _build_program(prep["schedule"], prep["e_pad"], repeat=repeat)
    return prep, nc, _build_in_maps(prep)


def kernel(x, edge_index, edge_attr, edge_len_emb, W1, W2, _results_out=None):
    prep, nc, in_maps = _prepare(x, edge_index, edge_attr, edge_len_emb, W1, W2)

    from concourse.bass_utils import run_bass_kernel_spmd

    res = run_bass_kernel_spmd(nc, in_maps, core_ids=list(range(NC)))
    if _results_out is not None:
        _results_out.append(res)

    return _postprocess([res.results[c]["out"] for c in range(NC)],
                        prep["assign"])

